# revision 25
# baseline (speedup 1.0000x reference)
"""Trainium2 Bass kernel for nn_Attention_54778012893268.

Fused QKV projection + RoPE + non-causal SDPA + output projection.
B=4, T=2048, C=2048, H=16, D=128, fp32 in / bf16 partial out.

Sharding: 8 cores = (batch b, head-group hg) pairs; b = core//2, hg = core%2.
Each core handles one batch's tokens and 8 of the 16 heads end-to-end
(tensor-parallel over heads for the projections), producing a partial
[T, C] bf16 output; the host upcasts and sums the two head-group
partials per batch.

Design (v7, 745us vs the 882us fp32r baseline; measured on HW traces):
- every matmul operand is bf16: the PE streams N=512 matmuls at the
  same 216ns floor regardless of dtype, but bf16 halves DMA/SBUF
  traffic and enables FWL (LDWEIGHTS 164->98ns). PSUM stays f32.
  Measured rel err 7.9e-3 vs the 2e-2 gate (bf16 RoPE + bf16 tree).
- x, q, k, v, ynorm all SBUF-resident end-to-end; no DRAM scratch.
- softmax denominator: instead of 512 ones-matmuls (116us of PE), a
  bf16 pairwise DVE tree sums the 16 E tiles per chunk pair and ONE
  ones-matmul per chunk (32 total) does the partition reduction.
- exp runs on chunk pairs: scores land in [128, 2x512] two-bank PSUM
  tiles so ACT issues 256 x [128,1024] exps (1.12us each) instead of
  512 x [128,512] (0.69us) - the attention phase is ACT-bound at
  ~267us, and the PE stream is ordered so ACT never starves:
  - each pair's denominator matmuls + recip + normalize are deferred
    into the NEXT pair (kt==3) so the PE FIFO never head-of-line
    blocks on the DVE tree tail,
  - the next pair's first two score groups are emitted inside kt 14/15
    of the current pair, ahead of av15 which waits on exp15,
  - y PSUM banks are freed early via bf16 copies so the next pair's AV
    accumulation never waits on recip/mul.
- DMA: only SP and ACT can issue HW DMAs, and queue throughput scales
  with descriptor row size (256B rows ~44GB/s, 1KB ~120GB/s per
  queue). All tensors are host-pre-tiled so every DMA is >=1KB-row
  contiguous, and loads alternate between the two queues. Weight
  halves are sequenced so each fi-group's stationary tiles arrive
  exactly one compute-burst ahead (fi 0-3 read half 0, fi 4-7 half 1);
  wv half 0 prefetches into a fresh pool during the k phase.

Layouts inside a core:
  x        [NCH, KT, 128, 512] bf16 (chunk-tiled, host-transposed)
  wq/wk/wv [KT, 2, 128, 512]   bf16 (half tiles, 1KB rows)
  q,k      resident [h][ci] tiles [128 d, 512 t] bf16 (RoPE'd)
  v        resident [vc][ti] tiles [128 t, 512 f(4 heads)] bf16
  scoresT  [128 k, 1024 q] f32 PSUM (keys on partitions, chunk pair)
  E        [128 k, 1024 q] bf16
  ynorm    resident [h] [128 d, 2048 t] bf16
  out      [T, C] bf16 partial (upcast + summed across paired cores
           on host)
"""

import math
import sys

import numpy as np

sys.path.insert(0, "/opt/trn_rl_repo")

P = 128
T = 2048
C = 2048
HPC = 8          # heads per core
D = 128
CH = 512         # T-chunk (PSUM bank width at fp32)
NCH = T // CH    # 4
KT = C // P      # 16 contraction tiles
TT = T // P      # 16 token tiles
SCALE = 1.0 / math.sqrt(D)
ROPE_BASE = 10000.0

_CACHED_NC = None


def build_nc():
    import concourse.bass as bass
    import concourse.tile as tile
    from concourse import bacc, mybir

    F32 = mybir.dt.float32
    BF16 = mybir.dt.bfloat16
    ts = bass.ts

    nc = bacc.Bacc("TRN2", target_bir_lowering=False, debug=False, num_devices=8)

    xtp = nc.dram_tensor("xtp", [NCH, KT, P, CH], BF16, kind="ExternalInput").ap()
    wqd = nc.dram_tensor("wqd", [KT, 2, P, CH], BF16, kind="ExternalInput").ap()
    wkd = nc.dram_tensor("wkd", [KT, 2, P, CH], BF16, kind="ExternalInput").ap()
    wvd = nc.dram_tensor("wvd", [KT, 2, P, CH], BF16, kind="ExternalInput").ap()
    wpd = nc.dram_tensor("wpd", [HPC, P, C], BF16, kind="ExternalInput").ap()
    cosm = nc.dram_tensor("cosm", [P, T], BF16, kind="ExternalInput").ap()
    sinm = nc.dram_tensor("sinm", [P, T], BF16, kind="ExternalInput").ap()
    onesd = nc.dram_tensor("onesd", [P, P], BF16, kind="ExternalInput").ap()
    out = nc.dram_tensor("out", [T, C], BF16, kind="ExternalOutput").ap()

    # pair-swap shuffle mask (within each 32-partition quadrant)
    SWAP_MASK = [i ^ 1 for i in range(32)]

    with tile.TileContext(nc) as tc:
        from contextlib import ExitStack

        with ExitStack() as outer:
            cpool = outer.enter_context(tc.tile_pool(name="const", bufs=1))
            qkres = outer.enter_context(tc.tile_pool(name="qkres", bufs=1))

            ones = cpool.tile([P, P], BF16, tag="ones")
            nc.sync.dma_start(ones[:], onesd)

            # ---------------- Phase 1: projections ----------------
            es1 = ExitStack()
            xpool = es1.enter_context(tc.tile_pool(name="xch", bufs=NCH * KT))
            wv0p = es1.enter_context(tc.tile_pool(name="wv0", bufs=1))
            rp = es1.enter_context(tc.tile_pool(name="rope", bufs=3))
            ps1 = es1.enter_context(tc.tile_pool(name="ps1", bufs=4, space="PSUM"))
            psv = es1.enter_context(tc.tile_pool(name="psv", bufs=4, space="PSUM"))
            # q/k-only pools, closed before the v phase to free SBUF
            es_qk = ExitStack()
            mpool = es_qk.enter_context(tc.tile_pool(name="masks", bufs=1))
            wpool = es_qk.enter_context(tc.tile_pool(name="w", bufs=2 * KT))

            cos_sb = mpool.tile([P, T], BF16, tag="cos")
            sin_sb = mpool.tile([P, T], BF16, tag="sin")

            # warm the PE HAM during the initial DMA ramp with junk matmuls;
            # the junk exp preloads the ACT function table before phase 2
            warm_ps = ps1.tile([P, 64], F32, tag="mm", name="warmps")
            for wi in range(100):
                nc.tensor.matmul(warm_ps[:], ones[:], ones[:, :64],
                                 start=(wi == 0), stop=(wi == 99))
            wexp = rp.tile([P, 64], BF16, tag="r0", name="warmexp")
            nc.scalar.activation(wexp[:], warm_ps[:],
                                 mybir.ActivationFunctionType.Exp, scale=SCALE)

            # x stays fully SBUF-resident for all of phase 1 (8MB bf16);
            # chunk DMAs are emitted one chunk ahead of use
            xtiles = {}

            def load_x(ci):
                for kt in range(KT):
                    xtl = xpool.tile([P, CH], BF16, tag="x",
                                     name=f"x{ci}_{kt}")
                    (nc.sync if kt % 2 == 0 else nc.scalar).dma_start(
                        xtl[:], xtp[ci, kt])
                    xtiles[ci, kt] = xtl

            q_t = {}   # (h, ci) -> [128 d, 512 t] bf16
            k_t = {}
            wvt = {}

            def load_w(wt, w_dram, half, phase):
                # one [128, 512] tile per (kt, half): 1KB rows; fi groups
                # 0-3 read half 0, fi 4-7 read half 1, so each half's DMA
                # burst pipelines exactly with the previous half's compute
                for kt in range(KT):
                    w0 = wpool.tile([P, CH], BF16, tag="w",
                                    name=f"w{kt}_{half}_{phase}")
                    (nc.scalar if kt % 2 == 0 else nc.sync).dma_start(
                        w0[:], w_dram[kt, half])
                    wt[kt, half] = w0

            wts = ({}, {})
            load_x(0)
            load_w(wts[0], wqd, 0, 0)
            nc.scalar.dma_start(cos_sb[:], cosm)
            nc.sync.dma_start(sin_sb[:], sinm)
            load_w(wts[0], wqd, 1, 0)
            load_x(1)
            load_x(2)
            load_x(3)

            for phase, (w_dram, dst) in enumerate(((wqd, q_t), (wkd, k_t))):
                wt = wts[phase]
                chunk_order = (0, 1, 2, 3) if phase == 0 else (3, 2, 1, 0)
                for nci, ci in enumerate(chunk_order):
                    for fi in range(HPC):
                        ps = ps1.tile([P, CH], F32, tag="mm")
                        for kt in range(KT):
                            nc.tensor.matmul(
                                ps[:],
                                wt[kt, fi // 4][:, ts(fi % 4, P)],
                                xtiles[ci, kt][:],
                                start=(kt == 0),
                                stop=(kt == KT - 1),
                            )
                        # prefetch next phase's weights during the last chunk:
                        # half 0 after the fi=3 group, half 1 after fi=7
                        if phase == 0 and nci == NCH - 1 and fi in (3, 7):
                            load_w(wts[1], wkd, fi // 4, 1)
                        # prefetch the v projection's half-0 weights into a
                        # fresh pool during k (no ring-slot waits)
                        if phase == 1 and nci == 0 and fi == 7:
                            for kt2 in range(KT):
                                wtl = wv0p.tile([P, CH], BF16,
                                                tag=f"wv0_{kt2}")
                                (nc.scalar if kt2 % 2 == 0
                                 else nc.sync).dma_start(wtl[:], wvd[kt2, 0])
                                wvt[kt2, 0] = wtl
                        # RoPE: one f32->bf16 copy, then 2x-rate bf16 ops
                        e0 = rp.tile([P, CH], BF16, tag="r0")
                        nc.vector.tensor_copy(e0[:], ps[:])
                        e1 = rp.tile([P, CH], BF16, tag="r1")
                        nc.vector.stream_shuffle(e1[:], e0[:], SWAP_MASK)
                        a = rp.tile([P, CH], BF16, tag="ra")
                        nc.vector.tensor_mul(a[:], e0[:], cos_sb[:, ts(ci, CH)])
                        b = rp.tile([P, CH], BF16, tag="rb")
                        nc.vector.tensor_mul(b[:], e1[:], sin_sb[:, ts(ci, CH)])
                        ro = qkres.tile([P, CH], BF16, tag=f"{'qk'[phase]}{fi}_{ci}")
                        nc.vector.tensor_add(ro[:], a[:], b[:])
                        dst[fi, ci] = ro

            # ---------------- Phase 1b: V projection ----------------
            es_qk.close()
            vres = outer.enter_context(
                tc.tile_pool(name="vres", bufs=1, side="right"))
            wvpool = es1.enter_context(tc.tile_pool(name="wv", bufs=KT))

            def load_wv1():
                for kt in range(KT):
                    wtl = wvpool.tile([P, CH], BF16, tag="wv",
                                      name=f"wv{kt}_1")
                    (nc.scalar if kt % 2 == 0 else nc.sync).dma_start(
                        wtl[:], wvd[kt, 1])
                    wvt[kt, 1] = wtl

            v_t = {}   # (vc, ti) -> [128 t, 512 f] bf16
            for vc in range(2):
                for ci in (0, 1, 2, 3):
                    for sub in range(4):
                        ti = 4 * ci + sub
                        if vc == 0 and ti == 0:
                            load_wv1()
                        ps = psv.tile([P, CH], F32, tag="mmv")
                        for kt in range(KT):
                            nc.tensor.matmul(
                                ps[:],
                                xtiles[ci, kt][:, ts(sub, P)],
                                wvt[kt, vc][:],
                                start=(kt == 0),
                                stop=(kt == KT - 1),
                            )
                        sb = vres.tile([P, CH], BF16, tag=f"v{vc}_{ti}")
                        nc.vector.tensor_copy(sb[:], ps[:])
                        v_t[vc, ti] = sb
            es1.close()

            # ---------------- Phase 2: attention ----------------
            ynp = outer.enter_context(tc.tile_pool(name="ynorm", bufs=1))
            wpp = outer.enter_context(tc.tile_pool(name="wp", bufs=1))
            ynorm = [ynp.tile([P, T], BF16, tag=f"yn{h}", name=f"ynorm{h}")
                     for h in range(HPC)]
            wpt = []
            for h in range(HPC):
                wtl = wpp.tile([P, C], BF16, tag=f"wp{h}", name=f"wpt{h}")
                (nc.sync if h % 2 == 0 else nc.scalar).dma_start(
                    wtl[:], wpd[h])
                wpt.append(wtl)

            with tc.tile_pool(name="ee", bufs=5) as ep, \
                 tc.tile_pool(name="st", bufs=9) as spool, \
                 tc.tile_pool(name="rc", bufs=2) as rcp, \
                 tc.tile_pool(name="yc", bufs=4) as ycp, \
                 tc.tile_pool(name="psS", bufs=2, space="PSUM") as psS, \
                 tc.tile_pool(name="psY", bufs=3, space="PSUM") as psY, \
                 tc.tile_pool(name="psD", bufs=1, space="PSUM") as psD:

                def emit_tail_half(t, j):
                    # denominator matmul + normalization of a finished pair,
                    # deferred into the NEXT pair (kt==3 and kt==5) so the PE
                    # FIFO never blocks on this pair's DVE tree tail while
                    # ACT starves; split so psD needs only one bank
                    h_, c0_, c1_, sfin, yc0, yc1 = t
                    cj = c0_ if j == 0 else c1_
                    ycj = yc0 if j == 0 else yc1
                    d = psD.tile([P, CH], F32, tag="d", name=f"d{j}")
                    nc.tensor.matmul(d[:], ones[:], sfin[:, ts(j, CH)],
                                     start=True, stop=True)
                    r = rcp.tile([P, CH], F32, tag="rc")
                    nc.vector.reciprocal_approx_fast(r[:], d[:])
                    nc.vector.tensor_mul(ynorm[h_][:, ts(cj, CH)], ycj[:], r[:])

                pairs = [(h, cp) for h in range(HPC) for cp in range(2)]
                s_store = {pi: {} for pi in range(len(pairs))}

                def s_mm(pi, kt):
                    h, cp = pairs[pi]
                    sp = psS.tile([P, 2 * CH], F32, tag="s", name=f"s{kt}")
                    kT = k_t[h, kt // 4][:, ts(kt % 4, P)]
                    nc.tensor.matmul(sp[:, 0:CH], kT, q_t[h, 2 * cp][:],
                                     start=True, stop=True)
                    nc.tensor.matmul(sp[:, CH:2 * CH], kT, q_t[h, 2 * cp + 1][:],
                                     start=True, stop=True)
                    return sp

                pend = None
                s_store[0][0] = s_mm(0, 0)
                s_store[0][1] = s_mm(0, 1)
                for pi, (h, cp) in enumerate(pairs):
                    vc, vo = h // 4, (h % 4) * P
                    c0, c1 = 2 * cp, 2 * cp + 1
                    if True:
                        y0 = psY.tile([P, CH], F32, tag="y", name="y0")
                        y1 = psY.tile([P, CH], F32, tag="y", name="y1")

                        s_tiles = s_store[pi]
                        es = {}
                        lvl = {}   # tree partial sums

                        for kt in range(TT):
                            if kt == 3 and pend is not None:
                                emit_tail_half(pend, 0)
                            if kt == 5 and pend is not None:
                                emit_tail_half(pend, 1)
                                pend = None
                            e = ep.tile([P, 2 * CH], BF16, tag="e")
                            nc.scalar.activation(
                                e[:], s_tiles.pop(kt)[:],
                                mybir.ActivationFunctionType.Exp, scale=SCALE,
                            )
                            es[kt] = e
                            if kt + 2 < TT:
                                s_tiles[kt + 2] = s_mm(pi, kt + 2)
                            elif pi + 1 < len(pairs):
                                # emit the NEXT pair's first score groups here
                                # so the tail of this pair (av15 waiting on
                                # exp15) never head-of-line-blocks them
                                s_store[pi + 1][kt + 2 - TT] = \
                                    s_mm(pi + 1, kt + 2 - TT)
                            vT = v_t[vc, kt][:, vo:vo + P]
                            nc.tensor.matmul(y0[:], vT, e[:, 0:CH],
                                             start=(kt == 0), stop=(kt == TT - 1))
                            nc.tensor.matmul(y1[:], vT, e[:, CH:2 * CH],
                                             start=(kt == 0), stop=(kt == TT - 1))
                            # denominator tree: bf16 pairwise adds on DVE
                            # (kt==15's adds are deferred below the yc copies
                            # so the y banks free as early as possible)
                            def tree_step(kt):
                                if kt % 2 == 1:
                                    t1 = spool.tile([P, 2 * CH], BF16, tag="t")
                                    nc.vector.tensor_add(
                                        t1[:], es.pop(kt - 1)[:], es.pop(kt)[:])
                                    lvl[1, kt // 2] = t1
                                for L in (1, 2, 3):
                                    j = (kt + 1) // (1 << (L + 1))
                                    if (kt + 1) % (1 << (L + 1)) == 0:
                                        t2 = spool.tile([P, 2 * CH], BF16,
                                                        tag="t")
                                        nc.vector.tensor_add(
                                            t2[:], lvl.pop((L, 2 * j - 2))[:],
                                            lvl.pop((L, 2 * j - 1))[:])
                                        lvl[L + 1, j - 1] = t2
                            if kt < TT - 1:
                                tree_step(kt)
                        # free the y psum banks early so the next pair's AV
                        # accumulation never waits on this pair's recip/mul
                        yc0 = ycp.tile([P, CH], BF16, tag="yc", name="yc0")
                        nc.vector.tensor_copy(yc0[:], y0[:])
                        yc1 = ycp.tile([P, CH], BF16, tag="yc", name="yc1")
                        nc.vector.tensor_copy(yc1[:], y1[:])
                        tree_step(TT - 1)
                        sfin = lvl.pop((4, 0))
                        pend = (h, c0, c1, sfin, yc0, yc1)
                emit_tail_half(pend, 0)
                emit_tail_half(pend, 1)

            # ---------------- Phase 3: output projection ----------------
            with tc.tile_pool(name="ost", bufs=4) as op, \
                 tc.tile_pool(name="ps3", bufs=4, space="PSUM") as ps3:
                for ti in range(TT):
                    for oc in range(NCH):
                        ps = ps3.tile([P, CH], F32, tag="mm3")
                        for h in range(HPC):
                            nc.tensor.matmul(
                                ps[:],
                                ynorm[h][:, ts(ti, P)],
                                wpt[h][:, ts(oc, CH)],
                                start=(h == 0),
                                stop=(h == HPC - 1),
                            )
                        ob = op.tile([P, CH], BF16, tag="ob")
                        nc.vector.tensor_copy(ob[:], ps[:])
                        (nc.sync if oc % 2 == 0 else nc.scalar).dma_start(
                            out[ts(ti, P), ts(oc, CH)], ob[:])

    nc.compile()
    return nc


def get_nc():
    global _CACHED_NC
    if _CACHED_NC is None:
        _CACHED_NC = build_nc()
    return _CACHED_NC


def make_rope_masks():
    half = D // 2
    inv = 1.0 / (ROPE_BASE ** (np.arange(half, dtype=np.float64) * 2.0 / D))
    ang = np.arange(T, dtype=np.float64)[:, None] * inv[None, :]  # [T, half]
    cos = np.cos(ang).T.astype(np.float32)  # [half, T]
    sin = np.sin(ang).T.astype(np.float32)
    cosm = np.empty((P, T), np.float32)
    sinm = np.empty((P, T), np.float32)
    cosm[0::2] = cos
    cosm[1::2] = cos
    sinm[0::2] = -sin
    sinm[1::2] = sin
    return cosm, sinm


def make_in_maps(x, w_attn, w_proj):
    import ml_dtypes
    BF = ml_dtypes.bfloat16

    x = np.asarray(x, dtype=np.float32)
    w_attn = np.asarray(w_attn, dtype=np.float32)
    w_proj = np.asarray(w_proj, dtype=np.float32)
    cosm, sinm = make_rope_masks()
    cosm16 = cosm.astype(BF)
    sinm16 = sinm.astype(BF)
    ones16 = np.ones((P, P), BF)
    in_maps = []
    for core in range(8):
        b, hg = core // 2, core % 2
        h0 = hg * HPC
        rq = slice(h0 * D, (h0 + HPC) * D)
        rk = slice(C + h0 * D, C + (h0 + HPC) * D)
        rv = slice(2 * C + h0 * D, 2 * C + (h0 + HPC) * D)
        # x tiles: [NCH, KT, P, CH] from x[b].T
        xt = np.ascontiguousarray(x[b].T.astype(BF))
        xtp = np.ascontiguousarray(
            xt.reshape(KT, P, NCH, CH).transpose(2, 0, 1, 3))
        # wq/wk/wv: [C, HPC*D] -> [KT, 2, P, CH] (1KB-row half tiles)
        def wtile(w):
            wT = w.T.astype(BF)  # [C, HPC*D]
            return np.ascontiguousarray(
                wT.reshape(KT, P, 2, CH).transpose(0, 2, 1, 3))
        wvd = wtile(w_attn[rv])
        wpT = np.ascontiguousarray(
            w_proj[:, h0 * D:(h0 + HPC) * D].T.astype(BF)).reshape(HPC, P, C)
        in_maps.append({
            "xtp": xtp,
            "wqd": wtile(w_attn[rq]),
            "wkd": wtile(w_attn[rk]),
            "wvd": wvd,
            "wpd": wpT,
            "cosm": cosm16,
            "sinm": sinm16,
            "onesd": ones16,
        })
    return in_maps


def combine_outputs(results):
    B = 4
    out = np.empty((B, T, C), np.float32)
    for b in range(B):
        out[b] = (results[2 * b]["out"].astype(np.float32)
                  + results[2 * b + 1]["out"].astype(np.float32))
    return out


def kernel(x, w_attn, w_proj):
    from concourse.bass_utils import run_bass_kernel_spmd

    nc = get_nc()
    in_maps = make_in_maps(x, w_attn, w_proj)
    res = run_bass_kernel_spmd(nc, in_maps, list(range(8)))
    return combine_outputs(res.results)


# revision 26
# speedup vs baseline: 1.1795x; 1.1795x over previous
"""Trainium2 Bass kernel for nn_Attention_54778012893268.

Fused QKV projection + RoPE + non-causal SDPA + output projection.
B=4, T=2048, C=2048, H=16, D=128, fp32 in / bf16 partial out.

Sharding: 8 cores = (batch b, head-group hg) pairs; b = core//2, hg = core%2.
Each core handles one batch's tokens and 8 of the 16 heads end-to-end
(tensor-parallel over heads for the projections), producing a partial
[T, C] bf16 output; the host upcasts and sums the two head-group
partials per batch.

Design (v7, 745us vs the 882us fp32r baseline; measured on HW traces):
- every matmul operand is bf16: the PE streams N=512 matmuls at the
  same 216ns floor regardless of dtype, but bf16 halves DMA/SBUF
  traffic and enables FWL (LDWEIGHTS 164->98ns). PSUM stays f32.
  Measured rel err 7.9e-3 vs the 2e-2 gate (bf16 RoPE + bf16 tree).
- x, q, k, v, ynorm all SBUF-resident end-to-end; no DRAM scratch.
- softmax denominator: instead of 512 ones-matmuls (116us of PE), a
  bf16 pairwise DVE tree sums the 16 E tiles per chunk pair and ONE
  ones-matmul per chunk (32 total) does the partition reduction.
- exp runs on chunk pairs: scores land in [128, 2x512] two-bank PSUM
  tiles so ACT issues 256 x [128,1024] exps (1.12us each) instead of
  512 x [128,512] (0.69us) - the attention phase is ACT-bound at
  ~267us, and the PE stream is ordered so ACT never starves:
  - each pair's denominator matmuls + recip + normalize are deferred
    into the NEXT pair (kt==3) so the PE FIFO never head-of-line
    blocks on the DVE tree tail,
  - the next pair's first two score groups are emitted inside kt 14/15
    of the current pair, ahead of av15 which waits on exp15,
  - y PSUM banks are freed early via bf16 copies so the next pair's AV
    accumulation never waits on recip/mul.
- DMA: only SP and ACT can issue HW DMAs, and queue throughput scales
  with descriptor row size (256B rows ~44GB/s, 1KB ~120GB/s per
  queue). All tensors are host-pre-tiled so every DMA is >=1KB-row
  contiguous, and loads alternate between the two queues. Weight
  halves are sequenced so each fi-group's stationary tiles arrive
  exactly one compute-burst ahead (fi 0-3 read half 0, fi 4-7 half 1);
  wv half 0 prefetches into a fresh pool during the k phase.

Layouts inside a core:
  x        [NCH, KT, 128, 512] bf16 (chunk-tiled, host-transposed)
  wq/wk/wv [KT, 2, 128, 512]   bf16 (half tiles, 1KB rows)
  q,k      resident [h][ci] tiles [128 d, 512 t] bf16 (RoPE'd)
  v        resident [vc][ti] tiles [128 t, 512 f(4 heads)] bf16
  scoresT  [128 k, 1024 q] f32 PSUM (keys on partitions, chunk pair)
  E        [128 k, 1024 q] bf16
  ynorm    resident [h] [128 d, 2048 t] bf16
  out      [T, C] bf16 partial (upcast + summed across paired cores
           on host)
"""

import math
import sys

import numpy as np

sys.path.insert(0, "/opt/trn_rl_repo")

P = 128
T = 2048
C = 2048
HPC = 8          # heads per core
D = 128
CH = 512         # T-chunk (PSUM bank width at fp32)
NCH = T // CH    # 4
KT = C // P      # 16 contraction tiles
TT = T // P      # 16 token tiles
SCALE = 1.0 / math.sqrt(D)
ROPE_BASE = 10000.0

_CACHED_NC = None


def build_nc():
    import concourse.bass as bass
    import concourse.tile as tile
    from concourse import bacc, mybir

    F32 = mybir.dt.float32
    BF16 = mybir.dt.bfloat16
    ts = bass.ts

    nc = bacc.Bacc("TRN2", target_bir_lowering=False, debug=False, num_devices=8)

    xtp = nc.dram_tensor("xtp", [NCH, KT, P, CH], BF16, kind="ExternalInput").ap()
    wqd = nc.dram_tensor("wqd", [KT, 2, P, CH], BF16, kind="ExternalInput").ap()
    wkd = nc.dram_tensor("wkd", [KT, 2, P, CH], BF16, kind="ExternalInput").ap()
    wvd = nc.dram_tensor("wvd", [KT, 2, P, CH], BF16, kind="ExternalInput").ap()
    wpd = nc.dram_tensor("wpd", [HPC, P, C], BF16, kind="ExternalInput").ap()
    cosm = nc.dram_tensor("cosm", [P, T], BF16, kind="ExternalInput").ap()
    sinm = nc.dram_tensor("sinm", [P, T], BF16, kind="ExternalInput").ap()
    onesd = nc.dram_tensor("onesd", [P, P], BF16, kind="ExternalInput").ap()
    out = nc.dram_tensor("out", [T, C], BF16, kind="ExternalOutput").ap()

    # pair-swap shuffle mask (within each 32-partition quadrant)
    SWAP_MASK = [i ^ 1 for i in range(32)]

    with tile.TileContext(nc) as tc:
        from contextlib import ExitStack

        with ExitStack() as outer:
            cpool = outer.enter_context(tc.tile_pool(name="const", bufs=1))
            qkres = outer.enter_context(tc.tile_pool(name="qkres", bufs=1))

            ones = cpool.tile([P, P], BF16, tag="ones")
            nc.sync.dma_start(ones[:], onesd)

            # ---------------- Phase 1: projections ----------------
            es1 = ExitStack()
            xpool = es1.enter_context(tc.tile_pool(name="xch", bufs=NCH * KT))
            wv0p = es1.enter_context(tc.tile_pool(name="wv0", bufs=1))
            rp = es1.enter_context(tc.tile_pool(name="rope", bufs=3))
            ps1 = es1.enter_context(tc.tile_pool(name="ps1", bufs=4, space="PSUM"))
            psv = es1.enter_context(tc.tile_pool(name="psv", bufs=4, space="PSUM"))
            # q/k-only pools, closed before the v phase to free SBUF
            es_qk = ExitStack()
            mpool = es_qk.enter_context(tc.tile_pool(name="masks", bufs=1))
            wpool = es_qk.enter_context(tc.tile_pool(name="w", bufs=2 * KT))

            cos_sb = mpool.tile([P, T], BF16, tag="cos")
            sin_sb = mpool.tile([P, T], BF16, tag="sin")

            # warm the PE HAM during the initial DMA ramp with junk matmuls;
            # the junk exp preloads the ACT function table before phase 2
            warm_ps = ps1.tile([P, 64], F32, tag="mm", name="warmps")
            for wi in range(100):
                nc.tensor.matmul(warm_ps[:], ones[:], ones[:, :64],
                                 start=(wi == 0), stop=(wi == 99))
            wexp = rp.tile([P, 64], BF16, tag="r0", name="warmexp")
            nc.scalar.activation(wexp[:], warm_ps[:],
                                 mybir.ActivationFunctionType.Exp, scale=SCALE)

            # x stays fully SBUF-resident for all of phase 1 (8MB bf16);
            # chunk DMAs are emitted one chunk ahead of use
            xtiles = {}

            def load_x(ci):
                for kt in range(KT):
                    xtl = xpool.tile([P, CH], BF16, tag="x",
                                     name=f"x{ci}_{kt}")
                    (nc.sync if kt % 2 == 0 else nc.scalar).dma_start(
                        xtl[:], xtp[ci, kt])
                    xtiles[ci, kt] = xtl

            q_t = {}   # (h, ci) -> [128 d, 512 t] bf16
            k_t = {}
            wvt = {}

            def load_w(wt, w_dram, half, phase):
                # one [128, 512] tile per (kt, half): 1KB rows; fi groups
                # 0-3 read half 0, fi 4-7 read half 1, so each half's DMA
                # burst pipelines exactly with the previous half's compute
                for kt in range(KT):
                    w0 = wpool.tile([P, CH], BF16, tag="w",
                                    name=f"w{kt}_{half}_{phase}")
                    (nc.scalar if kt % 2 == 0 else nc.sync).dma_start(
                        w0[:], w_dram[kt, half])
                    wt[kt, half] = w0

            wts = ({}, {})
            load_x(0)
            load_w(wts[0], wqd, 0, 0)
            nc.scalar.dma_start(cos_sb[:], cosm)
            nc.sync.dma_start(sin_sb[:], sinm)
            load_w(wts[0], wqd, 1, 0)
            load_x(1)
            load_x(2)
            load_x(3)

            for phase, (w_dram, dst) in enumerate(((wqd, q_t), (wkd, k_t))):
                wt = wts[phase]
                chunk_order = (0, 1, 2, 3) if phase == 0 else (3, 2, 1, 0)
                for nci, ci in enumerate(chunk_order):
                    for fi in range(HPC):
                        ps = ps1.tile([P, CH], F32, tag="mm")
                        for kt in range(KT):
                            nc.tensor.matmul(
                                ps[:],
                                wt[kt, fi // 4][:, ts(fi % 4, P)],
                                xtiles[ci, kt][:],
                                start=(kt == 0),
                                stop=(kt == KT - 1),
                            )
                        # prefetch next phase's weights during the last chunk:
                        # half 0 after the fi=3 group, half 1 after fi=7
                        if phase == 0 and nci == NCH - 1 and fi in (3, 7):
                            load_w(wts[1], wkd, fi // 4, 1)
                        # prefetch the v projection's half-0 weights into a
                        # fresh pool during k (no ring-slot waits)
                        if phase == 1 and nci == 0 and fi == 7:
                            for kt2 in range(KT):
                                wtl = wv0p.tile([P, CH], BF16,
                                                tag=f"wv0_{kt2}")
                                (nc.scalar if kt2 % 2 == 0
                                 else nc.sync).dma_start(wtl[:], wvd[kt2, 0])
                                wvt[kt2, 0] = wtl
                        # RoPE: one f32->bf16 copy, then 2x-rate bf16 ops
                        e0 = rp.tile([P, CH], BF16, tag="r0")
                        nc.vector.tensor_copy(e0[:], ps[:])
                        e1 = rp.tile([P, CH], BF16, tag="r1")
                        nc.vector.stream_shuffle(e1[:], e0[:], SWAP_MASK)
                        a = rp.tile([P, CH], BF16, tag="ra")
                        nc.vector.tensor_mul(a[:], e0[:], cos_sb[:, ts(ci, CH)])
                        b = rp.tile([P, CH], BF16, tag="rb")
                        nc.vector.tensor_mul(b[:], e1[:], sin_sb[:, ts(ci, CH)])
                        ro = qkres.tile([P, CH], BF16, tag=f"{'qk'[phase]}{fi}_{ci}")
                        nc.vector.tensor_add(ro[:], a[:], b[:])
                        dst[fi, ci] = ro

            # ---------------- Phase 1b: V projection ----------------
            es_qk.close()
            vres = outer.enter_context(
                tc.tile_pool(name="vres", bufs=1, side="right"))
            wvpool = es1.enter_context(tc.tile_pool(name="wv", bufs=KT))

            def load_wv1():
                for kt in range(KT):
                    wtl = wvpool.tile([P, CH], BF16, tag="wv",
                                      name=f"wv{kt}_1")
                    (nc.scalar if kt % 2 == 0 else nc.sync).dma_start(
                        wtl[:], wvd[kt, 1])
                    wvt[kt, 1] = wtl

            v_t = {}   # (vc, ti) -> [128 t, 512 f] bf16
            for vc in range(2):
                for ci in (0, 1, 2, 3):
                    for sub in range(4):
                        ti = 4 * ci + sub
                        if vc == 0 and ti == 0:
                            load_wv1()
                        ps = psv.tile([P, CH], F32, tag="mmv")
                        for kt in range(KT):
                            nc.tensor.matmul(
                                ps[:],
                                xtiles[ci, kt][:, ts(sub, P)],
                                wvt[kt, vc][:],
                                start=(kt == 0),
                                stop=(kt == KT - 1),
                            )
                        sb = vres.tile([P, CH], BF16, tag=f"v{vc}_{ti}")
                        nc.vector.tensor_copy(sb[:], ps[:])
                        v_t[vc, ti] = sb
            es1.close()

            # ---------------- Phase 2: attention ----------------
            ynp = outer.enter_context(tc.tile_pool(name="ynorm", bufs=1))
            wpp = outer.enter_context(tc.tile_pool(name="wp", bufs=1))
            ynorm = [ynp.tile([P, T], BF16, tag=f"yn{h}", name=f"ynorm{h}")
                     for h in range(HPC)]
            wpt = []
            for h in range(HPC):
                wtl = wpp.tile([P, C], BF16, tag=f"wp{h}", name=f"wpt{h}")
                (nc.sync if h % 2 == 0 else nc.scalar).dma_start(
                    wtl[:], wpd[h])
                wpt.append(wtl)

            with tc.tile_pool(name="ee", bufs=5) as ep, \
                 tc.tile_pool(name="st", bufs=9) as spool, \
                 tc.tile_pool(name="rc", bufs=2) as rcp, \
                 tc.tile_pool(name="yc", bufs=4) as ycp, \
                 tc.tile_pool(name="psS", bufs=2, space="PSUM") as psS, \
                 tc.tile_pool(name="psY", bufs=2, space="PSUM") as psY, \
                 tc.tile_pool(name="psD", bufs=2, space="PSUM") as psD:

                def emit_tail(t):
                    # denominator matmuls + normalization of a finished pair,
                    # deferred into the NEXT pair so the PE FIFO never blocks
                    # on this pair's DVE tree tail while ACT starves
                    h_, c0_, c1_, sfin, yc0, yc1 = t
                    d0 = psD.tile([P, CH], F32, tag="d", name="d0")
                    nc.tensor.matmul(d0[:], ones[:], sfin[:, 0:CH],
                                     start=True, stop=True)
                    d1 = psD.tile([P, CH], F32, tag="d", name="d1")
                    nc.tensor.matmul(d1[:], ones[:], sfin[:, CH:2 * CH],
                                     start=True, stop=True)
                    r0 = rcp.tile([P, CH], F32, tag="rc")
                    nc.vector.reciprocal_approx_fast(r0[:], d0[:])
                    nc.vector.tensor_mul(ynorm[h_][:, ts(c0_, CH)], yc0[:], r0[:])
                    r1 = rcp.tile([P, CH], F32, tag="rc")
                    nc.vector.reciprocal_approx_fast(r1[:], d1[:])
                    nc.vector.tensor_mul(ynorm[h_][:, ts(c1_, CH)], yc1[:], r1[:])

                pairs = [(h, cp) for h in range(HPC) for cp in range(2)]
                s_store = {pi: {} for pi in range(len(pairs))}

                def s_mm(pi, kt):
                    h, cp = pairs[pi]
                    sp = psS.tile([P, 2 * CH], F32, tag="s", name=f"s{kt}")
                    kT = k_t[h, kt // 4][:, ts(kt % 4, P)]
                    nc.tensor.matmul(sp[:, 0:CH], kT, q_t[h, 2 * cp][:],
                                     start=True, stop=True)
                    nc.tensor.matmul(sp[:, CH:2 * CH], kT, q_t[h, 2 * cp + 1][:],
                                     start=True, stop=True)
                    return sp

                pend = None
                s_store[0][0] = s_mm(0, 0)
                s_store[0][1] = s_mm(0, 1)
                for pi, (h, cp) in enumerate(pairs):
                    vc, vo = h // 4, (h % 4) * P
                    c0, c1 = 2 * cp, 2 * cp + 1
                    if True:
                        y0 = psY.tile([P, CH], F32, tag="y", name="y0")
                        y1 = psY.tile([P, CH], F32, tag="y", name="y1")

                        s_tiles = s_store[pi]
                        es = {}
                        lvl = {}   # tree partial sums

                        for kt in range(TT):
                            if kt == 3 and pend is not None:
                                emit_tail(pend)
                                pend = None
                            e = ep.tile([P, 2 * CH], BF16, tag="e")
                            nc.scalar.activation(
                                e[:], s_tiles.pop(kt)[:],
                                mybir.ActivationFunctionType.Exp, scale=SCALE,
                            )
                            es[kt] = e
                            if kt + 2 < TT:
                                s_tiles[kt + 2] = s_mm(pi, kt + 2)
                            elif pi + 1 < len(pairs):
                                # emit the NEXT pair's first score groups here
                                # so the tail of this pair (av15 waiting on
                                # exp15) never head-of-line-blocks them
                                s_store[pi + 1][kt + 2 - TT] = \
                                    s_mm(pi + 1, kt + 2 - TT)
                            vT = v_t[vc, kt][:, vo:vo + P]
                            nc.tensor.matmul(y0[:], vT, e[:, 0:CH],
                                             start=(kt == 0), stop=(kt == TT - 1))
                            nc.tensor.matmul(y1[:], vT, e[:, CH:2 * CH],
                                             start=(kt == 0), stop=(kt == TT - 1))
                            # denominator tree: bf16 pairwise adds on DVE
                            if kt % 2 == 1:
                                t1 = spool.tile([P, 2 * CH], BF16, tag="t")
                                nc.vector.tensor_add(t1[:], es.pop(kt - 1)[:],
                                                     es.pop(kt)[:])
                                lvl[1, kt // 2] = t1
                            for L in (1, 2, 3):
                                j = (kt + 1) // (1 << (L + 1))
                                if (kt + 1) % (1 << (L + 1)) == 0:
                                    t2 = spool.tile([P, 2 * CH], BF16, tag="t")
                                    nc.vector.tensor_add(
                                        t2[:], lvl.pop((L, 2 * j - 2))[:],
                                        lvl.pop((L, 2 * j - 1))[:])
                                    lvl[L + 1, j - 1] = t2
                        sfin = lvl.pop((4, 0))
                        # free the y psum banks early so the next pair's AV
                        # accumulation never waits on this pair's recip/mul
                        yc0 = ycp.tile([P, CH], BF16, tag="yc", name="yc0")
                        nc.vector.tensor_copy(yc0[:], y0[:])
                        yc1 = ycp.tile([P, CH], BF16, tag="yc", name="yc1")
                        nc.vector.tensor_copy(yc1[:], y1[:])
                        pend = (h, c0, c1, sfin, yc0, yc1)
                emit_tail(pend)

            # ---------------- Phase 3: output projection ----------------
            with tc.tile_pool(name="ost", bufs=4) as op, \
                 tc.tile_pool(name="ps3", bufs=4, space="PSUM") as ps3:
                for ti in range(TT):
                    for oc in range(NCH):
                        ps = ps3.tile([P, CH], F32, tag="mm3")
                        for h in range(HPC):
                            nc.tensor.matmul(
                                ps[:],
                                ynorm[h][:, ts(ti, P)],
                                wpt[h][:, ts(oc, CH)],
                                start=(h == 0),
                                stop=(h == HPC - 1),
                            )
                        ob = op.tile([P, CH], BF16, tag="ob")
                        nc.vector.tensor_copy(ob[:], ps[:])
                        (nc.sync if oc % 2 == 0 else nc.scalar).dma_start(
                            out[ts(ti, P), ts(oc, CH)], ob[:])

    nc.compile()
    return nc


def get_nc():
    global _CACHED_NC
    if _CACHED_NC is None:
        _CACHED_NC = build_nc()
    return _CACHED_NC


def make_rope_masks():
    half = D // 2
    inv = 1.0 / (ROPE_BASE ** (np.arange(half, dtype=np.float64) * 2.0 / D))
    ang = np.arange(T, dtype=np.float64)[:, None] * inv[None, :]  # [T, half]
    cos = np.cos(ang).T.astype(np.float32)  # [half, T]
    sin = np.sin(ang).T.astype(np.float32)
    cosm = np.empty((P, T), np.float32)
    sinm = np.empty((P, T), np.float32)
    cosm[0::2] = cos
    cosm[1::2] = cos
    sinm[0::2] = -sin
    sinm[1::2] = sin
    return cosm, sinm


def make_in_maps(x, w_attn, w_proj):
    import ml_dtypes
    BF = ml_dtypes.bfloat16

    x = np.asarray(x, dtype=np.float32)
    w_attn = np.asarray(w_attn, dtype=np.float32)
    w_proj = np.asarray(w_proj, dtype=np.float32)
    cosm, sinm = make_rope_masks()
    cosm16 = cosm.astype(BF)
    sinm16 = sinm.astype(BF)
    ones16 = np.ones((P, P), BF)
    in_maps = []
    for core in range(8):
        b, hg = core // 2, core % 2
        h0 = hg * HPC
        rq = slice(h0 * D, (h0 + HPC) * D)
        rk = slice(C + h0 * D, C + (h0 + HPC) * D)
        rv = slice(2 * C + h0 * D, 2 * C + (h0 + HPC) * D)
        # x tiles: [NCH, KT, P, CH] from x[b].T
        xt = np.ascontiguousarray(x[b].T.astype(BF))
        xtp = np.ascontiguousarray(
            xt.reshape(KT, P, NCH, CH).transpose(2, 0, 1, 3))
        # wq/wk/wv: [C, HPC*D] -> [KT, 2, P, CH] (1KB-row half tiles)
        def wtile(w):
            wT = w.T.astype(BF)  # [C, HPC*D]
            return np.ascontiguousarray(
                wT.reshape(KT, P, 2, CH).transpose(0, 2, 1, 3))
        wvd = wtile(w_attn[rv])
        wpT = np.ascontiguousarray(
            w_proj[:, h0 * D:(h0 + HPC) * D].T.astype(BF)).reshape(HPC, P, C)
        in_maps.append({
            "xtp": xtp,
            "wqd": wtile(w_attn[rq]),
            "wkd": wtile(w_attn[rk]),
            "wvd": wvd,
            "wpd": wpT,
            "cosm": cosm16,
            "sinm": sinm16,
            "onesd": ones16,
        })
    return in_maps


def combine_outputs(results):
    B = 4
    out = np.empty((B, T, C), np.float32)
    for b in range(B):
        out[b] = (results[2 * b]["out"].astype(np.float32)
                  + results[2 * b + 1]["out"].astype(np.float32))
    return out


def kernel(x, w_attn, w_proj):
    from concourse.bass_utils import run_bass_kernel_spmd

    nc = get_nc()
    in_maps = make_in_maps(x, w_attn, w_proj)
    res = run_bass_kernel_spmd(nc, in_maps, list(range(8)))
    return combine_outputs(res.results)
